# revision 6
# baseline (speedup 1.0000x reference)
"""Trainium2 Bass kernel for nn_MultiHeadAttention_82523501625370.

Multi-head attention (B=2, S=2048, D=2048, H=16, Dh=128) with RoPE and a
causal mask (the reference's sliding-window clause `(j-i) >= 1024` is a
subset of the causal clause `j > i`, so the effective mask is plain causal).

Sharding: 8 cores = 2 batches x 4 head-groups (4 heads each).
Per core: column-parallel Wq/Wk/Wv (512 output dims), local attention for
its 4 heads, row-parallel Wo slice -> partial [S, D] output, summed on host.

On-chip dataflow (per core):
  - projections: Q^T/K^T/V^T tiles via fp32r matmuls (W^T chunks streamed,
    x^T s-block resident), RoPE applied via a constant J-permutation matmul
    plus DVE multiplies, results stored bf16.
  - V^T transposed to V [s, dh] layout via TensorE (bf16).
  - scores: S[sq,sk] = Q^T.T @ K^T per 128-row query tile, additive causal
    mask on the diagonal chunk, exp on ScalarE with fused 1/sqrt(dh) scale
    and fused row-sum (accum_out), P stored bf16, normalized by reciprocal
    row-sum on GPSIMD.
  - P@V: P chunks transposed on TensorE (bf16), out^T accumulated in PSUM.
  - output projection: attnT^T @ Wo^T slice (bf16), partial [S, D] fp32.
"""

import math
import sys

import numpy as np

sys.path.insert(0, "/opt/trn_rl_repo")

import ml_dtypes  # noqa: E402

import concourse.bass as bass  # noqa: E402
from concourse import bacc  # noqa: E402
import concourse.mybir as mybir  # noqa: E402
import concourse.tile as tile  # noqa: E402
from concourse import bass_utils  # noqa: E402

F32 = mybir.dt.float32
F32R = mybir.dt.float32r
BF16 = mybir.dt.bfloat16
AF = mybir.ActivationFunctionType
BF16NP = ml_dtypes.bfloat16

B, S, D = 2, 2048, 2048
H_LOCAL = 4          # heads per core
DH = 128
HD_LOCAL = H_LOCAL * DH   # 512
N_CORES = 8
N_SQ = S // 128      # 16 query tiles
N_SB = 4             # s-blocks for projection (512 each)
SBLK = S // N_SB     # 512
KCH = D // 128       # 16 contraction chunks
SCALE = 1.0 / math.sqrt(DH)
NEG = -30000.0

_CACHE = {}
_RUN_KWARGS = {}  # test harness can set e.g. {"trace": True} for profiling


def _build_program():
    if "nc" in _CACHE:
        return _CACHE["nc"]

    nc = bacc.Bacc("TRN2", target_bir_lowering=False, debug=False)

    xT_d = nc.dram_tensor("xT", (D, S), F32R, kind="ExternalInput").ap()
    wqT_d = nc.dram_tensor("wqT", (D, HD_LOCAL), F32R, kind="ExternalInput").ap()
    wkT_d = nc.dram_tensor("wkT", (D, HD_LOCAL), F32R, kind="ExternalInput").ap()
    wvT_d = nc.dram_tensor("wvT", (D, HD_LOCAL), F32R, kind="ExternalInput").ap()
    woT_d = nc.dram_tensor("woT", (HD_LOCAL, D), BF16, kind="ExternalInput").ap()
    cosT_d = nc.dram_tensor("cosT", (DH, S), BF16, kind="ExternalInput").ap()
    sinT_d = nc.dram_tensor("sinT", (DH, S), BF16, kind="ExternalInput").ap()
    jmat_d = nc.dram_tensor("jmatT", (DH, DH), F32R, kind="ExternalInput").ap()
    maskL_d = nc.dram_tensor("maskL", (128, 128), F32, kind="ExternalInput").ap()
    ident_d = nc.dram_tensor("ident", (128, 128), BF16, kind="ExternalInput").ap()
    out_d = nc.dram_tensor("out", (S, D), F32, kind="ExternalOutput").ap()

    with tile.TileContext(nc) as tc:
        with tc.tile_pool(name="resident", bufs=1) as res:
            qt_sb = res.tile([128, H_LOCAL * S], BF16)       # (h, s)
            kt_sb = res.tile([128, H_LOCAL * S], BF16)       # (h, s)
            v_sb = res.tile([128, N_SQ * HD_LOCAL], BF16)    # (s-chunk, hd)
            at_sb = res.tile([128, H_LOCAL * S], BF16)       # attnT (h, s)
            wo_sb = res.tile([128, H_LOCAL * S], BF16)       # (hc, e)
            cos_sb = res.tile([128, S], BF16)
            sin_sb = res.tile([128, S], BF16)
            jmat_sb = res.tile([128, 128], F32R)
            maskL_sb = res.tile([128, 128], F32)
            ident_sb = res.tile([128, 128], BF16)

            nc.sync.dma_start(cos_sb[:], cosT_d[:])
            nc.sync.dma_start(sin_sb[:], sinT_d[:])
            nc.sync.dma_start(jmat_sb[:], jmat_d[:])
            nc.sync.dma_start(maskL_sb[:], maskL_d[:])
            nc.sync.dma_start(ident_sb[:], ident_d[:])
            for hc in range(H_LOCAL):
                nc.sync.dma_start(
                    wo_sb[:, hc * S:(hc + 1) * S], woT_d[hc * 128:(hc + 1) * 128, :]
                )

            # ---------------- projections + RoPE ----------------
            with (
                tc.tile_pool(name="xpool", bufs=2) as xpool,
                tc.tile_pool(name="wpool", bufs=3) as wpool,
                tc.tile_pool(name="pstage", bufs=3) as pstage,
                tc.tile_pool(name="ptmp", bufs=2) as ptmp,
                tc.tile_pool(name="projps", bufs=1, space="PSUM") as projps,
                tc.tile_pool(name="rotps", bufs=2, space="PSUM") as rotps,
                tc.tile_pool(name="vtps", bufs=1, space="PSUM") as vtps,
            ):
                for sb in range(N_SB):
                    xblk = xpool.tile([128, KCH * SBLK], F32R, tag="xblk")
                    for k in range(KCH):
                        nc.sync.dma_start(
                            xblk[:, k * SBLK:(k + 1) * SBLK],
                            xT_d[k * 128:(k + 1) * 128, sb * SBLK:(sb + 1) * SBLK],
                        )
                    for wT_dram, kind in ((wqT_d, "q"), (wkT_d, "k"), (wvT_d, "v")):
                        psums = []
                        for j in range(H_LOCAL):
                            pj = projps.tile([128, SBLK], F32, tag="proj", bufs=4,
                                             name=f"proj_{sb}_{kind}_{j}")
                            psums.append(pj)
                        for k in range(KCH):
                            wt = wpool.tile([128, HD_LOCAL], F32R, tag="w",
                                            name=f"w_{sb}_{kind}_{k}")
                            nc.sync.dma_start(wt[:], wT_dram[k * 128:(k + 1) * 128, :])
                            for j in range(H_LOCAL):
                                nc.tensor.matmul(
                                    psums[j][:],
                                    wt[:, j * 128:(j + 1) * 128],
                                    xblk[:, k * SBLK:(k + 1) * SBLK],
                                    start=(k == 0),
                                    stop=(k == KCH - 1),
                                )
                        for j in range(H_LOCAL):
                            if kind in ("q", "k"):
                                # RoPE: q' = q*cos + (J q)*sin, J applied on PE
                                qst = pstage.tile([128, SBLK], F32R, tag="qst",
                                                  name=f"qst_{sb}_{kind}_{j}")
                                nc.scalar.copy(qst[:], psums[j][:])
                                rps = rotps.tile([128, SBLK], F32, tag="rot",
                                                 name=f"rot_{sb}_{kind}_{j}")
                                nc.tensor.matmul(
                                    rps[:], jmat_sb[:], qst[:], start=True, stop=True,
                                )
                                m1 = ptmp.tile([128, SBLK], F32, tag="m1",
                                               name=f"m1_{sb}_{kind}_{j}")
                                nc.vector.tensor_mul(
                                    m1[:], qst[:], cos_sb[:, sb * SBLK:(sb + 1) * SBLK]
                                )
                                m2 = ptmp.tile([128, SBLK], F32, tag="m2",
                                               name=f"m2_{sb}_{kind}_{j}")
                                nc.vector.tensor_mul(
                                    m2[:], rps[:], sin_sb[:, sb * SBLK:(sb + 1) * SBLK]
                                )
                                dest = qt_sb if kind == "q" else kt_sb
                                nc.vector.tensor_add(
                                    dest[:, j * S + sb * SBLK: j * S + (sb + 1) * SBLK],
                                    m1[:], m2[:],
                                )
                            else:
                                # V^T -> V via TensorE transpose (bf16)
                                vst = pstage.tile([128, SBLK], BF16, tag="vst",
                                                  name=f"vst_{sb}_{j}")
                                nc.scalar.copy(vst[:], psums[j][:])
                                for ss in range(SBLK // 128):
                                    vt = vtps.tile([128, 128], BF16, tag="vt",
                                                   bufs=2, name=f"vt_{sb}_{j}_{ss}")
                                    nc.tensor.transpose(
                                        vt[:], vst[:, ss * 128:(ss + 1) * 128],
                                        ident_sb[:],
                                    )
                                    c = sb * (SBLK // 128) + ss
                                    nc.scalar.copy(
                                        v_sb[:, c * HD_LOCAL + j * 128:
                                             c * HD_LOCAL + (j + 1) * 128],
                                        vt[:],
                                    )

            # ---------------- attention ----------------
            with (
                tc.tile_pool(name="ppool", bufs=3) as ppool,
                tc.tile_pool(name="accpool", bufs=8) as accpool,
                tc.tile_pool(name="ptsb", bufs=6) as ptsbp,
                tc.tile_pool(name="scps", bufs=2, space="PSUM") as scps,
                tc.tile_pool(name="ptps", bufs=3, space="PSUM") as ptps,
                tc.tile_pool(name="ops", bufs=2, space="PSUM") as ops,
            ):
                for h in range(H_LOCAL):
                    for t in range(N_SQ):
                        n = t + 1  # causal: chunks 0..t
                        nblk = (n + 3) // 4
                        p_t = ppool.tile([128, N_SQ * 128], BF16, tag="p",
                                         name=f"p_{h}_{t}")
                        accs = []
                        for bi in range(nblk):
                            w = min(4, n - 4 * bi) * 128
                            sps = scps.tile([128, 512], F32, tag="sc",
                                            name=f"sc_{h}_{t}_{bi}")
                            nc.tensor.matmul(
                                sps[:, :w],
                                qt_sb[:, h * S + t * 128: h * S + (t + 1) * 128],
                                kt_sb[:, h * S + bi * 512: h * S + bi * 512 + w],
                                start=True, stop=True,
                            )
                            # additive causal mask on the diagonal chunk (c == t)
                            if t // 4 == bi:
                                off = (t % 4) * 128
                                nc.vector.tensor_add(
                                    sps[:, off:off + 128], sps[:, off:off + 128],
                                    maskL_sb[:],
                                )
                            acc = accpool.tile([128, 1], F32, tag="acc",
                                               name=f"acc_{h}_{t}_{bi}")
                            nc.scalar.activation(
                                p_t[:, bi * 512: bi * 512 + w], sps[:, :w],
                                AF.Exp, scale=SCALE, accum_out=acc[:],
                            )
                            accs.append(acc)
                        rs = accs[0]
                        for acc in accs[1:]:
                            rs2 = accpool.tile([128, 1], F32, tag="rss",
                                               name=f"rs_{h}_{t}")
                            nc.vector.tensor_add(rs2[:], rs[:], acc[:])
                            rs = rs2
                        rcp = accpool.tile([128, 1], F32, tag="rcp",
                                           name=f"rcp_{h}_{t}")
                        nc.vector.reciprocal(rcp[:], rs[:])
                        nc.gpsimd.tensor_scalar_mul(
                            p_t[:, :n * 128], p_t[:, :n * 128], rcp[:]
                        )
                        opsum = ops.tile([128, 128], F32, tag="o",
                                         name=f"o_{h}_{t}")
                        for c in range(n):
                            ptp = ptps.tile([128, 128], BF16, tag="pt",
                                            name=f"pt_{h}_{t}_{c}")
                            nc.tensor.transpose(
                                ptp[:], p_t[:, c * 128:(c + 1) * 128], ident_sb[:]
                            )
                            pts = ptsbp.tile([128, 128], BF16, tag="pts",
                                             name=f"pts_{h}_{t}_{c}")
                            nc.scalar.copy(pts[:], ptp[:])
                            nc.tensor.matmul(
                                opsum[:],
                                v_sb[:, c * HD_LOCAL + h * 128:
                                     c * HD_LOCAL + (h + 1) * 128],
                                pts[:],
                                start=(c == 0), stop=(c == n - 1),
                            )
                        nc.scalar.copy(
                            at_sb[:, h * S + t * 128: h * S + (t + 1) * 128],
                            opsum[:],
                        )

            # ---------------- output projection ----------------
            with (
                tc.tile_pool(name="osb", bufs=3) as osbp,
                tc.tile_pool(name="fps", bufs=3, space="PSUM") as fps,
            ):
                for st in range(N_SQ):
                    for e in range(D // 512):
                        fp = fps.tile([128, 512], F32, tag="f",
                                      name=f"f_{st}_{e}")
                        for hc in range(H_LOCAL):
                            nc.tensor.matmul(
                                fp[:],
                                at_sb[:, hc * S + st * 128: hc * S + (st + 1) * 128],
                                wo_sb[:, hc * S + e * 512: hc * S + (e + 1) * 512],
                                start=(hc == 0), stop=(hc == H_LOCAL - 1),
                            )
                        ob = osbp.tile([128, 512], F32, tag="ob",
                                       name=f"ob_{st}_{e}")
                        nc.vector.tensor_copy(ob[:], fp[:])
                        nc.sync.dma_start(
                            out_d[st * 128:(st + 1) * 128, e * 512:(e + 1) * 512],
                            ob[:],
                        )

    nc.compile()
    _CACHE["nc"] = nc
    return nc


def _host_constants():
    if "consts" in _CACHE:
        return _CACHE["consts"]
    inv = 1.0 / (10000.0 ** (np.arange(0, DH, 2, dtype=np.float32) / DH))
    t = np.arange(S, dtype=np.float32)
    freqs = np.outer(t, inv)                       # [S, 64]
    emb = np.concatenate([freqs, freqs], -1)       # [S, 128]
    cosT = np.cos(emb).T.astype(BF16NP).copy()     # [128, S]
    sinT = np.sin(emb).T.astype(BF16NP).copy()
    j = np.zeros((DH, DH), dtype=np.float32)
    half = DH // 2
    for p in range(half):
        j[p, p + half] = -1.0
        j[p + half, p] = 1.0
    jmatT = j.T.copy()                             # lhsT so that lhsT.T = J
    r = np.arange(128)
    maskL = np.where(r[None, :] <= r[:, None], 0.0, NEG).astype(np.float32)
    ident = np.eye(128, dtype=BF16NP)
    _CACHE["consts"] = (cosT, sinT, jmatT, maskL, ident)
    return _CACHE["consts"]


def kernel(hidden_states, Wq, Wk, Wv, Wo):
    hidden_states = np.asarray(hidden_states, dtype=np.float32)
    Wq = np.asarray(Wq, dtype=np.float32)
    Wk = np.asarray(Wk, dtype=np.float32)
    Wv = np.asarray(Wv, dtype=np.float32)
    Wo = np.asarray(Wo, dtype=np.float32)

    nc = _build_program()
    cosT, sinT, jmatT, maskL, ident = _host_constants()

    in_maps = []
    for core in range(N_CORES):
        b, g = divmod(core, N_CORES // B)
        hd0 = g * HD_LOCAL
        in_maps.append({
            "xT": np.ascontiguousarray(hidden_states[b].T),
            "wqT": np.ascontiguousarray(Wq[hd0:hd0 + HD_LOCAL, :].T),
            "wkT": np.ascontiguousarray(Wk[hd0:hd0 + HD_LOCAL, :].T),
            "wvT": np.ascontiguousarray(Wv[hd0:hd0 + HD_LOCAL, :].T),
            "woT": np.ascontiguousarray(Wo[:, hd0:hd0 + HD_LOCAL].T).astype(BF16NP),
            "cosT": cosT,
            "sinT": sinT,
            "jmatT": jmatT,
            "maskL": maskL,
            "ident": ident,
        })

    res = bass_utils.run_bass_kernel_spmd(
        nc, in_maps, core_ids=list(range(N_CORES)), **_RUN_KWARGS
    )
    _CACHE["last_results"] = res

    out = np.zeros((B, S, D), dtype=np.float32)
    for core in range(N_CORES):
        b = core // (N_CORES // B)
        out[b] += res.results[core]["out"]
    return out


# revision 8
# speedup vs baseline: 2.2854x; 2.2854x over previous
"""Trainium2 Bass kernel for nn_MultiHeadAttention_82523501625370.

Multi-head attention (B=2, S=2048, D=2048, H=16, Dh=128) with RoPE and a
causal mask (the reference's sliding-window clause `(j-i) >= 1024` is a
subset of the causal clause `j > i`, so the effective mask is plain causal).

Sharding: 8 cores = 2 batches x 4 head-groups (4 heads each).
Per core: column-parallel Wq/Wk/Wv (512 output dims), local attention for
its 4 heads, row-parallel Wo slice -> partial [S, D] output, summed on host.

On-chip dataflow (per core):
  - projections: Q^T/K^T/V^T tiles via fp32r matmuls (W^T chunks streamed,
    x^T s-block resident), RoPE applied via a constant J-permutation matmul
    plus DVE multiplies, results stored bf16.
  - V^T transposed to V [s, dh] layout via TensorE (bf16).
  - scores: S[sq,sk] = Q^T.T @ K^T per 128-row query tile, additive causal
    mask on the diagonal chunk, exp on ScalarE with fused 1/sqrt(dh) scale
    and fused row-sum (accum_out), P stored bf16, normalized by reciprocal
    row-sum on GPSIMD.
  - P@V: P chunks transposed on TensorE (bf16), out^T accumulated in PSUM.
  - output projection: attnT^T @ Wo^T slice (bf16), partial [S, D] fp32.
"""

import math
import sys

import numpy as np

sys.path.insert(0, "/opt/trn_rl_repo")

import ml_dtypes  # noqa: E402

import concourse.bass as bass  # noqa: E402
from concourse import bacc  # noqa: E402
import concourse.mybir as mybir  # noqa: E402
import concourse.tile as tile  # noqa: E402
from concourse import bass_utils  # noqa: E402

F32 = mybir.dt.float32
F32R = mybir.dt.float32r
BF16 = mybir.dt.bfloat16
AF = mybir.ActivationFunctionType
BF16NP = ml_dtypes.bfloat16

B, S, D = 2, 2048, 2048
H_LOCAL = 4          # heads per core
DH = 128
HD_LOCAL = H_LOCAL * DH   # 512
N_CORES = 8
N_SQ = S // 128      # 16 query tiles
N_SB = 4             # s-blocks for projection (512 each)
SBLK = S // N_SB     # 512
KCH = D // 128       # 16 contraction chunks
SCALE = 1.0 / math.sqrt(DH)
NEG = -30000.0

_CACHE = {}
_RUN_KWARGS = {}  # test harness can set e.g. {"trace": True} for profiling


def _build_program():
    if "nc" in _CACHE:
        return _CACHE["nc"]

    nc = bacc.Bacc("TRN2", target_bir_lowering=False, debug=False)

    xT_d = nc.dram_tensor("xT", (D, S), F32R, kind="ExternalInput").ap()
    wqT_d = nc.dram_tensor("wqT", (D, HD_LOCAL), F32R, kind="ExternalInput").ap()
    wkT_d = nc.dram_tensor("wkT", (D, HD_LOCAL), F32R, kind="ExternalInput").ap()
    wvT_d = nc.dram_tensor("wvT", (D, HD_LOCAL), F32R, kind="ExternalInput").ap()
    woT_d = nc.dram_tensor("woT", (HD_LOCAL, D), BF16, kind="ExternalInput").ap()
    cosT_d = nc.dram_tensor("cosT", (DH, S), BF16, kind="ExternalInput").ap()
    sinT_d = nc.dram_tensor("sinT", (DH, S), BF16, kind="ExternalInput").ap()
    jmat_d = nc.dram_tensor("jmatT", (DH, DH), F32R, kind="ExternalInput").ap()
    maskL_d = nc.dram_tensor("maskL", (128, 128), F32, kind="ExternalInput").ap()
    ident_d = nc.dram_tensor("ident", (128, 128), BF16, kind="ExternalInput").ap()
    out_d = nc.dram_tensor("out", (S, D), F32, kind="ExternalOutput").ap()

    with tile.TileContext(nc) as tc:
        with tc.tile_pool(name="resident", bufs=1) as res:
            qt_sb = res.tile([128, H_LOCAL * S], BF16)       # (h, s)
            kt_sb = res.tile([128, H_LOCAL * S], BF16)       # (h, s)
            v_sb = res.tile([128, N_SQ * HD_LOCAL], BF16)    # (s-chunk, hd)
            at_sb = res.tile([128, H_LOCAL * S], BF16)       # attnT (h, s)
            wo_sb = res.tile([128, H_LOCAL * S], BF16)       # (hc, e)
            cos_sb = res.tile([128, S], BF16)
            sin_sb = res.tile([128, S], BF16)
            jmat_sb = res.tile([128, 128], F32R)
            maskL_sb = res.tile([128, 128], F32)
            ident_sb = res.tile([128, 128], BF16)

            nc.sync.dma_start(cos_sb[:], cosT_d[:])
            nc.sync.dma_start(sin_sb[:], sinT_d[:])
            nc.sync.dma_start(jmat_sb[:], jmat_d[:])
            nc.sync.dma_start(maskL_sb[:], maskL_d[:])
            nc.sync.dma_start(ident_sb[:], ident_d[:])
            for hc in range(H_LOCAL):
                nc.sync.dma_start(
                    wo_sb[:, hc * S:(hc + 1) * S], woT_d[hc * 128:(hc + 1) * 128, :]
                )

            # ---------------- projections + RoPE ----------------
            with (
                tc.tile_pool(name="xpool", bufs=2) as xpool,
                tc.tile_pool(name="wpool", bufs=3) as wpool,
                tc.tile_pool(name="pstage", bufs=3) as pstage,
                tc.tile_pool(name="ptmp", bufs=2) as ptmp,
                tc.tile_pool(name="projps", bufs=1, space="PSUM") as projps,
                tc.tile_pool(name="rotps", bufs=2, space="PSUM") as rotps,
                tc.tile_pool(name="vtps", bufs=1, space="PSUM") as vtps,
            ):
                for sb in range(N_SB):
                    xblk = xpool.tile([128, KCH * SBLK], F32R, tag="xblk")
                    for k in range(KCH):
                        nc.sync.dma_start(
                            xblk[:, k * SBLK:(k + 1) * SBLK],
                            xT_d[k * 128:(k + 1) * 128, sb * SBLK:(sb + 1) * SBLK],
                        )
                    for wT_dram, kind in ((wqT_d, "q"), (wkT_d, "k"), (wvT_d, "v")):
                        psums = []
                        for j in range(H_LOCAL):
                            pj = projps.tile([128, SBLK], F32, tag="proj", bufs=4,
                                             name=f"proj_{sb}_{kind}_{j}")
                            psums.append(pj)
                        for k in range(KCH):
                            wt = wpool.tile([128, HD_LOCAL], F32R, tag="w",
                                            name=f"w_{sb}_{kind}_{k}")
                            nc.sync.dma_start(wt[:], wT_dram[k * 128:(k + 1) * 128, :])
                            for j in range(H_LOCAL):
                                nc.tensor.matmul(
                                    psums[j][:],
                                    wt[:, j * 128:(j + 1) * 128],
                                    xblk[:, k * SBLK:(k + 1) * SBLK],
                                    start=(k == 0),
                                    stop=(k == KCH - 1),
                                )
                        for j in range(H_LOCAL):
                            if kind in ("q", "k"):
                                # RoPE: q' = q*cos + (J q)*sin, J applied on PE
                                qst = pstage.tile([128, SBLK], F32R, tag="qst",
                                                  name=f"qst_{sb}_{kind}_{j}")
                                nc.scalar.copy(qst[:], psums[j][:])
                                rps = rotps.tile([128, SBLK], F32, tag="rot",
                                                 name=f"rot_{sb}_{kind}_{j}")
                                nc.tensor.matmul(
                                    rps[:], jmat_sb[:], qst[:], start=True, stop=True,
                                )
                                m1 = ptmp.tile([128, SBLK], F32, tag="m1",
                                               name=f"m1_{sb}_{kind}_{j}")
                                nc.vector.tensor_mul(
                                    m1[:], qst[:], cos_sb[:, sb * SBLK:(sb + 1) * SBLK]
                                )
                                m2 = ptmp.tile([128, SBLK], F32, tag="m2",
                                               name=f"m2_{sb}_{kind}_{j}")
                                nc.vector.tensor_mul(
                                    m2[:], rps[:], sin_sb[:, sb * SBLK:(sb + 1) * SBLK]
                                )
                                dest = qt_sb if kind == "q" else kt_sb
                                nc.vector.tensor_add(
                                    dest[:, j * S + sb * SBLK: j * S + (sb + 1) * SBLK],
                                    m1[:], m2[:],
                                )
                            else:
                                # V^T -> V via TensorE transpose (bf16)
                                vst = pstage.tile([128, SBLK], BF16, tag="vst",
                                                  name=f"vst_{sb}_{j}")
                                nc.scalar.copy(vst[:], psums[j][:])
                                for ss in range(SBLK // 128):
                                    vt = vtps.tile([128, 128], BF16, tag="vt",
                                                   bufs=2, name=f"vt_{sb}_{j}_{ss}")
                                    nc.tensor.transpose(
                                        vt[:], vst[:, ss * 128:(ss + 1) * 128],
                                        ident_sb[:],
                                    )
                                    c = sb * (SBLK // 128) + ss
                                    nc.scalar.copy(
                                        v_sb[:, c * HD_LOCAL + j * 128:
                                             c * HD_LOCAL + (j + 1) * 128],
                                        vt[:],
                                    )

            # ---------------- attention ----------------
            with (
                tc.tile_pool(name="ppool", bufs=3) as ppool,
                tc.tile_pool(name="accpool", bufs=8) as accpool,
                tc.tile_pool(name="ptsb", bufs=6) as ptsbp,
                tc.tile_pool(name="scps", bufs=2, space="PSUM") as scps,
                tc.tile_pool(name="ptps", bufs=3, space="PSUM") as ptps,
                tc.tile_pool(name="ops", bufs=2, space="PSUM") as ops,
            ):
                for h in range(H_LOCAL):
                    for t in range(N_SQ):
                        n = t + 1  # causal: chunks 0..t
                        nblk = (n + 3) // 4
                        p_t = ppool.tile([128, N_SQ * 128], BF16, tag="p",
                                         name=f"p_{h}_{t}")
                        accs = []
                        for bi in range(nblk):
                            w = min(4, n - 4 * bi) * 128
                            sps = scps.tile([128, 512], F32, tag="sc",
                                            name=f"sc_{h}_{t}_{bi}")
                            nc.tensor.matmul(
                                sps[:, :w],
                                qt_sb[:, h * S + t * 128: h * S + (t + 1) * 128],
                                kt_sb[:, h * S + bi * 512: h * S + bi * 512 + w],
                                start=True, stop=True,
                            )
                            # additive causal mask on the diagonal chunk (c == t)
                            if t // 4 == bi:
                                off = (t % 4) * 128
                                nc.vector.tensor_add(
                                    sps[:, off:off + 128], sps[:, off:off + 128],
                                    maskL_sb[:],
                                )
                            acc = accpool.tile([128, 1], F32, tag="acc",
                                               name=f"acc_{h}_{t}_{bi}")
                            nc.scalar.activation(
                                p_t[:, bi * 512: bi * 512 + w], sps[:, :w],
                                AF.Exp, scale=SCALE, accum_out=acc[:],
                            )
                            accs.append(acc)
                        rs = accs[0]
                        for acc in accs[1:]:
                            rs2 = accpool.tile([128, 1], F32, tag="rss",
                                               name=f"rs_{h}_{t}")
                            nc.vector.tensor_add(rs2[:], rs[:], acc[:])
                            rs = rs2
                        rcp = accpool.tile([128, 1], F32, tag="rcp",
                                           name=f"rcp_{h}_{t}")
                        nc.vector.reciprocal(rcp[:], rs[:])
                        nc.scalar.mul(p_t[:, :n * 128], p_t[:, :n * 128], rcp[:])
                        opsum = ops.tile([128, 128], F32, tag="o",
                                         name=f"o_{h}_{t}")
                        for c in range(n):
                            ptp = ptps.tile([128, 128], BF16, tag="pt",
                                            name=f"pt_{h}_{t}_{c}")
                            nc.tensor.transpose(
                                ptp[:], p_t[:, c * 128:(c + 1) * 128], ident_sb[:]
                            )
                            pts = ptsbp.tile([128, 128], BF16, tag="pts",
                                             name=f"pts_{h}_{t}_{c}")
                            nc.vector.tensor_copy(pts[:], ptp[:])
                            nc.tensor.matmul(
                                opsum[:],
                                v_sb[:, c * HD_LOCAL + h * 128:
                                     c * HD_LOCAL + (h + 1) * 128],
                                pts[:],
                                start=(c == 0), stop=(c == n - 1),
                            )
                        nc.scalar.copy(
                            at_sb[:, h * S + t * 128: h * S + (t + 1) * 128],
                            opsum[:],
                        )

            # ---------------- output projection ----------------
            with (
                tc.tile_pool(name="osb", bufs=3) as osbp,
                tc.tile_pool(name="fps", bufs=3, space="PSUM") as fps,
            ):
                for st in range(N_SQ):
                    for e in range(D // 512):
                        fp = fps.tile([128, 512], F32, tag="f",
                                      name=f"f_{st}_{e}")
                        for hc in range(H_LOCAL):
                            nc.tensor.matmul(
                                fp[:],
                                at_sb[:, hc * S + st * 128: hc * S + (st + 1) * 128],
                                wo_sb[:, hc * S + e * 512: hc * S + (e + 1) * 512],
                                start=(hc == 0), stop=(hc == H_LOCAL - 1),
                            )
                        ob = osbp.tile([128, 512], F32, tag="ob",
                                       name=f"ob_{st}_{e}")
                        nc.vector.tensor_copy(ob[:], fp[:])
                        nc.sync.dma_start(
                            out_d[st * 128:(st + 1) * 128, e * 512:(e + 1) * 512],
                            ob[:],
                        )

    nc.compile()
    _CACHE["nc"] = nc
    return nc


def _host_constants():
    if "consts" in _CACHE:
        return _CACHE["consts"]
    inv = 1.0 / (10000.0 ** (np.arange(0, DH, 2, dtype=np.float32) / DH))
    t = np.arange(S, dtype=np.float32)
    freqs = np.outer(t, inv)                       # [S, 64]
    emb = np.concatenate([freqs, freqs], -1)       # [S, 128]
    cosT = np.cos(emb).T.astype(BF16NP).copy()     # [128, S]
    sinT = np.sin(emb).T.astype(BF16NP).copy()
    j = np.zeros((DH, DH), dtype=np.float32)
    half = DH // 2
    for p in range(half):
        j[p, p + half] = -1.0
        j[p + half, p] = 1.0
    jmatT = j.T.copy()                             # lhsT so that lhsT.T = J
    r = np.arange(128)
    maskL = np.where(r[None, :] <= r[:, None], 0.0, NEG).astype(np.float32)
    ident = np.eye(128, dtype=BF16NP)
    _CACHE["consts"] = (cosT, sinT, jmatT, maskL, ident)
    return _CACHE["consts"]


def kernel(hidden_states, Wq, Wk, Wv, Wo):
    hidden_states = np.asarray(hidden_states, dtype=np.float32)
    Wq = np.asarray(Wq, dtype=np.float32)
    Wk = np.asarray(Wk, dtype=np.float32)
    Wv = np.asarray(Wv, dtype=np.float32)
    Wo = np.asarray(Wo, dtype=np.float32)

    nc = _build_program()
    cosT, sinT, jmatT, maskL, ident = _host_constants()

    in_maps = []
    for core in range(N_CORES):
        b, g = divmod(core, N_CORES // B)
        hd0 = g * HD_LOCAL
        in_maps.append({
            "xT": np.ascontiguousarray(hidden_states[b].T),
            "wqT": np.ascontiguousarray(Wq[hd0:hd0 + HD_LOCAL, :].T),
            "wkT": np.ascontiguousarray(Wk[hd0:hd0 + HD_LOCAL, :].T),
            "wvT": np.ascontiguousarray(Wv[hd0:hd0 + HD_LOCAL, :].T),
            "woT": np.ascontiguousarray(Wo[:, hd0:hd0 + HD_LOCAL].T).astype(BF16NP),
            "cosT": cosT,
            "sinT": sinT,
            "jmatT": jmatT,
            "maskL": maskL,
            "ident": ident,
        })

    res = bass_utils.run_bass_kernel_spmd(
        nc, in_maps, core_ids=list(range(N_CORES)), **_RUN_KWARGS
    )
    _CACHE["last_results"] = res

    out = np.zeros((B, S, D), dtype=np.float32)
    for core in range(N_CORES):
        b = core // (N_CORES // B)
        out[b] += res.results[core]["out"]
    return out


# revision 9
# speedup vs baseline: 2.5283x; 1.1063x over previous
"""Trainium2 Bass kernel for nn_MultiHeadAttention_82523501625370.

Multi-head attention (B=2, S=2048, D=2048, H=16, Dh=128) with RoPE and a
causal mask (the reference's sliding-window clause `(j-i) >= 1024` is a
subset of the causal clause `j > i`, so the effective mask is plain causal).

Sharding: 8 cores = 2 batches x 4 head-groups (4 heads each).
Per core: column-parallel Wq/Wk/Wv (512 output dims), local attention for
its 4 heads, row-parallel Wo slice -> partial [S, D] output, summed on host.

On-chip dataflow (per core):
  - projections: Q^T/K^T/V^T tiles via fp32r matmuls (W^T chunks streamed,
    x^T s-block resident), RoPE applied via a constant J-permutation matmul
    plus DVE multiplies, results stored bf16.
  - V^T transposed to V [s, dh] layout via TensorE (bf16).
  - scores: S[sq,sk] = Q^T.T @ K^T per 128-row query tile, additive causal
    mask on the diagonal chunk, exp on ScalarE with fused 1/sqrt(dh) scale
    and fused row-sum (accum_out), P stored bf16, normalized by reciprocal
    row-sum on GPSIMD.
  - P@V: P chunks transposed on TensorE (bf16), out^T accumulated in PSUM.
  - output projection: attnT^T @ Wo^T slice (bf16), partial [S, D] fp32.
"""

import math
import sys

import numpy as np

sys.path.insert(0, "/opt/trn_rl_repo")

import ml_dtypes  # noqa: E402

import concourse.bass as bass  # noqa: E402
from concourse import bacc  # noqa: E402
import concourse.mybir as mybir  # noqa: E402
import concourse.tile as tile  # noqa: E402
from concourse import bass_utils  # noqa: E402

F32 = mybir.dt.float32
F32R = mybir.dt.float32r
BF16 = mybir.dt.bfloat16
AF = mybir.ActivationFunctionType
BF16NP = ml_dtypes.bfloat16

B, S, D = 2, 2048, 2048
H_LOCAL = 4          # heads per core
DH = 128
HD_LOCAL = H_LOCAL * DH   # 512
N_CORES = 8
N_SQ = S // 128      # 16 query tiles
N_SB = 4             # s-blocks for projection (512 each)
SBLK = S // N_SB     # 512
KCH = D // 128       # 16 contraction chunks
SCALE = 1.0 / math.sqrt(DH)
NEG = -30000.0

_CACHE = {}
_RUN_KWARGS = {}  # test harness can set e.g. {"trace": True} for profiling


def _build_program():
    if "nc" in _CACHE:
        return _CACHE["nc"]

    nc = bacc.Bacc("TRN2", target_bir_lowering=False, debug=False)

    xT_d = nc.dram_tensor("xT", (D, S), BF16, kind="ExternalInput").ap()
    wqT_d = nc.dram_tensor("wqT", (D, HD_LOCAL), BF16, kind="ExternalInput").ap()
    wkT_d = nc.dram_tensor("wkT", (D, HD_LOCAL), BF16, kind="ExternalInput").ap()
    wvT_d = nc.dram_tensor("wvT", (D, HD_LOCAL), BF16, kind="ExternalInput").ap()
    woT_d = nc.dram_tensor("woT", (HD_LOCAL, D), BF16, kind="ExternalInput").ap()
    cosT_d = nc.dram_tensor("cosT", (DH, S), BF16, kind="ExternalInput").ap()
    sinT_d = nc.dram_tensor("sinT", (DH, S), BF16, kind="ExternalInput").ap()
    jmat_d = nc.dram_tensor("jmatT", (DH, DH), BF16, kind="ExternalInput").ap()
    maskL_d = nc.dram_tensor("maskL", (128, 128), F32, kind="ExternalInput").ap()
    ident_d = nc.dram_tensor("ident", (128, 128), BF16, kind="ExternalInput").ap()
    out_d = nc.dram_tensor("out", (S, D), F32, kind="ExternalOutput").ap()

    with tile.TileContext(nc) as tc:
        with tc.tile_pool(name="resident", bufs=1) as res:
            qt_sb = res.tile([128, H_LOCAL * S], BF16)       # (h, s)
            kt_sb = res.tile([128, H_LOCAL * S], BF16)       # (h, s)
            v_sb = res.tile([128, N_SQ * HD_LOCAL], BF16)    # (s-chunk, hd)
            at_sb = res.tile([128, H_LOCAL * S], BF16)       # attnT (h, s)
            wo_sb = res.tile([128, H_LOCAL * S], BF16)       # (hc, e)
            cos_sb = res.tile([128, S], BF16)
            sin_sb = res.tile([128, S], BF16)
            jmat_sb = res.tile([128, 128], BF16)
            maskL_sb = res.tile([128, 128], F32)
            ident_sb = res.tile([128, 128], BF16)

            nc.sync.dma_start(cos_sb[:], cosT_d[:])
            nc.sync.dma_start(sin_sb[:], sinT_d[:])
            nc.sync.dma_start(jmat_sb[:], jmat_d[:])
            nc.sync.dma_start(maskL_sb[:], maskL_d[:])
            nc.sync.dma_start(ident_sb[:], ident_d[:])
            for hc in range(H_LOCAL):
                nc.sync.dma_start(
                    wo_sb[:, hc * S:(hc + 1) * S], woT_d[hc * 128:(hc + 1) * 128, :]
                )

            # ---------------- projections + RoPE ----------------
            with (
                tc.tile_pool(name="xpool", bufs=2) as xpool,
                tc.tile_pool(name="wpool", bufs=3) as wpool,
                tc.tile_pool(name="pstage", bufs=3) as pstage,
                tc.tile_pool(name="ptmp", bufs=2) as ptmp,
                tc.tile_pool(name="projps", bufs=1, space="PSUM") as projps,
                tc.tile_pool(name="rotps", bufs=2, space="PSUM") as rotps,
                tc.tile_pool(name="vtps", bufs=1, space="PSUM") as vtps,
            ):
                for sb in range(N_SB):
                    xblk = xpool.tile([128, KCH * SBLK], BF16, tag="xblk")
                    for k in range(KCH):
                        nc.sync.dma_start(
                            xblk[:, k * SBLK:(k + 1) * SBLK],
                            xT_d[k * 128:(k + 1) * 128, sb * SBLK:(sb + 1) * SBLK],
                        )
                    for wT_dram, kind in ((wqT_d, "q"), (wkT_d, "k"), (wvT_d, "v")):
                        psums = []
                        for j in range(H_LOCAL):
                            pj = projps.tile([128, SBLK], F32, tag="proj", bufs=4,
                                             name=f"proj_{sb}_{kind}_{j}")
                            psums.append(pj)
                        for k in range(KCH):
                            wt = wpool.tile([128, HD_LOCAL], BF16, tag="w",
                                            name=f"w_{sb}_{kind}_{k}")
                            nc.sync.dma_start(wt[:], wT_dram[k * 128:(k + 1) * 128, :])
                            for j in range(H_LOCAL):
                                nc.tensor.matmul(
                                    psums[j][:],
                                    wt[:, j * 128:(j + 1) * 128],
                                    xblk[:, k * SBLK:(k + 1) * SBLK],
                                    start=(k == 0),
                                    stop=(k == KCH - 1),
                                )
                        for j in range(H_LOCAL):
                            if kind in ("q", "k"):
                                # RoPE: q' = q*cos + (J q)*sin, J applied on PE
                                qst = pstage.tile([128, SBLK], BF16, tag="qst",
                                                  name=f"qst_{sb}_{kind}_{j}")
                                nc.scalar.copy(qst[:], psums[j][:])
                                rps = rotps.tile([128, SBLK], F32, tag="rot",
                                                 name=f"rot_{sb}_{kind}_{j}")
                                nc.tensor.matmul(
                                    rps[:], jmat_sb[:], qst[:], start=True, stop=True,
                                )
                                m1 = ptmp.tile([128, SBLK], F32, tag="m1",
                                               name=f"m1_{sb}_{kind}_{j}")
                                nc.vector.tensor_mul(
                                    m1[:], qst[:], cos_sb[:, sb * SBLK:(sb + 1) * SBLK]
                                )
                                m2 = ptmp.tile([128, SBLK], F32, tag="m2",
                                               name=f"m2_{sb}_{kind}_{j}")
                                nc.vector.tensor_mul(
                                    m2[:], rps[:], sin_sb[:, sb * SBLK:(sb + 1) * SBLK]
                                )
                                dest = qt_sb if kind == "q" else kt_sb
                                nc.vector.tensor_add(
                                    dest[:, j * S + sb * SBLK: j * S + (sb + 1) * SBLK],
                                    m1[:], m2[:],
                                )
                            else:
                                # V^T -> V via TensorE transpose (bf16)
                                vst = pstage.tile([128, SBLK], BF16, tag="vst",
                                                  name=f"vst_{sb}_{j}")
                                nc.scalar.copy(vst[:], psums[j][:])
                                for ss in range(SBLK // 128):
                                    vt = vtps.tile([128, 128], BF16, tag="vt",
                                                   bufs=2, name=f"vt_{sb}_{j}_{ss}")
                                    nc.tensor.transpose(
                                        vt[:], vst[:, ss * 128:(ss + 1) * 128],
                                        ident_sb[:],
                                    )
                                    c = sb * (SBLK // 128) + ss
                                    nc.scalar.copy(
                                        v_sb[:, c * HD_LOCAL + j * 128:
                                             c * HD_LOCAL + (j + 1) * 128],
                                        vt[:],
                                    )

            # ---------------- attention ----------------
            with (
                tc.tile_pool(name="ppool", bufs=3) as ppool,
                tc.tile_pool(name="accpool", bufs=8) as accpool,
                tc.tile_pool(name="ptsb", bufs=10) as ptsbp,
                tc.tile_pool(name="scps", bufs=2, space="PSUM") as scps,
                tc.tile_pool(name="ptps", bufs=4, space="PSUM") as ptps,
                tc.tile_pool(name="ops", bufs=2, space="PSUM") as ops,
            ):
                for h in range(H_LOCAL):
                    for t in range(N_SQ):
                        n = t + 1  # causal: chunks 0..t
                        nblk = (n + 3) // 4
                        p_t = ppool.tile([128, N_SQ * 128], BF16, tag="p",
                                         name=f"p_{h}_{t}")
                        accs = []
                        for bi in range(nblk):
                            w = min(4, n - 4 * bi) * 128
                            sps = scps.tile([128, 512], F32, tag="sc",
                                            name=f"sc_{h}_{t}_{bi}")
                            nc.tensor.matmul(
                                sps[:, :w],
                                qt_sb[:, h * S + t * 128: h * S + (t + 1) * 128],
                                kt_sb[:, h * S + bi * 512: h * S + bi * 512 + w],
                                start=True, stop=True,
                            )
                            # additive causal mask on the diagonal chunk (c == t)
                            if t // 4 == bi:
                                off = (t % 4) * 128
                                nc.vector.tensor_add(
                                    sps[:, off:off + 128], sps[:, off:off + 128],
                                    maskL_sb[:],
                                )
                            acc = accpool.tile([128, 1], F32, tag="acc",
                                               name=f"acc_{h}_{t}_{bi}")
                            nc.scalar.activation(
                                p_t[:, bi * 512: bi * 512 + w], sps[:, :w],
                                AF.Exp, scale=SCALE, accum_out=acc[:],
                            )
                            accs.append(acc)
                        rs = accs[0]
                        for acc in accs[1:]:
                            rs2 = accpool.tile([128, 1], F32, tag="rss",
                                               name=f"rs_{h}_{t}")
                            nc.vector.tensor_add(rs2[:], rs[:], acc[:])
                            rs = rs2
                        rcp = accpool.tile([128, 1], F32, tag="rcp",
                                           name=f"rcp_{h}_{t}")
                        nc.vector.reciprocal(rcp[:], rs[:])
                        nc.scalar.mul(p_t[:, :n * 128], p_t[:, :n * 128], rcp[:])
                        opsum = ops.tile([128, 128], F32, tag="o",
                                         name=f"o_{h}_{t}")
                        for c in range(n):
                            ptp = ptps.tile([128, 128], BF16, tag="pt",
                                            name=f"pt_{h}_{t}_{c}")
                            nc.tensor.transpose(
                                ptp[:], p_t[:, c * 128:(c + 1) * 128], ident_sb[:]
                            )
                            pts = ptsbp.tile([128, 128], BF16, tag="pts",
                                             name=f"pts_{h}_{t}_{c}")
                            nc.vector.tensor_copy(pts[:], ptp[:])
                            nc.tensor.matmul(
                                opsum[:],
                                v_sb[:, c * HD_LOCAL + h * 128:
                                     c * HD_LOCAL + (h + 1) * 128],
                                pts[:],
                                start=(c == 0), stop=(c == n - 1),
                            )
                        nc.scalar.copy(
                            at_sb[:, h * S + t * 128: h * S + (t + 1) * 128],
                            opsum[:],
                        )

            # ---------------- output projection ----------------
            with (
                tc.tile_pool(name="osb", bufs=3) as osbp,
                tc.tile_pool(name="fps", bufs=3, space="PSUM") as fps,
            ):
                for st in range(N_SQ):
                    for e in range(D // 512):
                        fp = fps.tile([128, 512], F32, tag="f",
                                      name=f"f_{st}_{e}")
                        for hc in range(H_LOCAL):
                            nc.tensor.matmul(
                                fp[:],
                                at_sb[:, hc * S + st * 128: hc * S + (st + 1) * 128],
                                wo_sb[:, hc * S + e * 512: hc * S + (e + 1) * 512],
                                start=(hc == 0), stop=(hc == H_LOCAL - 1),
                            )
                        ob = osbp.tile([128, 512], F32, tag="ob",
                                       name=f"ob_{st}_{e}")
                        nc.vector.tensor_copy(ob[:], fp[:])
                        nc.sync.dma_start(
                            out_d[st * 128:(st + 1) * 128, e * 512:(e + 1) * 512],
                            ob[:],
                        )

    nc.compile()
    _CACHE["nc"] = nc
    return nc


def _host_constants():
    if "consts" in _CACHE:
        return _CACHE["consts"]
    inv = 1.0 / (10000.0 ** (np.arange(0, DH, 2, dtype=np.float32) / DH))
    t = np.arange(S, dtype=np.float32)
    freqs = np.outer(t, inv)                       # [S, 64]
    emb = np.concatenate([freqs, freqs], -1)       # [S, 128]
    cosT = np.cos(emb).T.astype(BF16NP).copy()     # [128, S]
    sinT = np.sin(emb).T.astype(BF16NP).copy()
    j = np.zeros((DH, DH), dtype=np.float32)
    half = DH // 2
    for p in range(half):
        j[p, p + half] = -1.0
        j[p + half, p] = 1.0
    jmatT = j.T.astype(BF16NP).copy()                             # lhsT so that lhsT.T = J
    r = np.arange(128)
    maskL = np.where(r[None, :] <= r[:, None], 0.0, NEG).astype(np.float32)
    ident = np.eye(128, dtype=BF16NP)
    _CACHE["consts"] = (cosT, sinT, jmatT, maskL, ident)
    return _CACHE["consts"]


def kernel(hidden_states, Wq, Wk, Wv, Wo):
    hidden_states = np.asarray(hidden_states, dtype=np.float32)
    Wq = np.asarray(Wq, dtype=np.float32)
    Wk = np.asarray(Wk, dtype=np.float32)
    Wv = np.asarray(Wv, dtype=np.float32)
    Wo = np.asarray(Wo, dtype=np.float32)

    nc = _build_program()
    cosT, sinT, jmatT, maskL, ident = _host_constants()

    in_maps = []
    for core in range(N_CORES):
        b, g = divmod(core, N_CORES // B)
        hd0 = g * HD_LOCAL
        in_maps.append({
            "xT": np.ascontiguousarray(hidden_states[b].T).astype(BF16NP),
            "wqT": np.ascontiguousarray(Wq[hd0:hd0 + HD_LOCAL, :].T).astype(BF16NP),
            "wkT": np.ascontiguousarray(Wk[hd0:hd0 + HD_LOCAL, :].T).astype(BF16NP),
            "wvT": np.ascontiguousarray(Wv[hd0:hd0 + HD_LOCAL, :].T).astype(BF16NP),
            "woT": np.ascontiguousarray(Wo[:, hd0:hd0 + HD_LOCAL].T).astype(BF16NP),
            "cosT": cosT,
            "sinT": sinT,
            "jmatT": jmatT,
            "maskL": maskL,
            "ident": ident,
        })

    res = bass_utils.run_bass_kernel_spmd(
        nc, in_maps, core_ids=list(range(N_CORES)), **_RUN_KWARGS
    )
    _CACHE["last_results"] = res

    out = np.zeros((B, S, D), dtype=np.float32)
    for core in range(N_CORES):
        b = core // (N_CORES // B)
        out[b] += res.results[core]["out"]
    return out


# revision 10
# speedup vs baseline: 3.0328x; 1.1995x over previous
"""Trainium2 Bass kernel for nn_MultiHeadAttention_82523501625370.

Multi-head attention (B=2, S=2048, D=2048, H=16, Dh=128) with RoPE and a
causal mask (the reference's sliding-window clause `(j-i) >= 1024` is a
subset of the causal clause `j > i`, so the effective mask is plain causal).

Sharding: 8 cores = 2 batches x 4 head-groups (4 heads each).
Per core: column-parallel Wq/Wk/Wv (512 output dims), local attention for
its 4 heads, row-parallel Wo slice -> partial [S, D] output, summed on host.

On-chip dataflow (per core):
  - projections: Q^T/K^T/V^T tiles via fp32r matmuls (W^T chunks streamed,
    x^T s-block resident), RoPE applied via a constant J-permutation matmul
    plus DVE multiplies, results stored bf16.
  - V^T transposed to V [s, dh] layout via TensorE (bf16).
  - scores: S[sq,sk] = Q^T.T @ K^T per 128-row query tile, additive causal
    mask on the diagonal chunk, exp on ScalarE with fused 1/sqrt(dh) scale
    and fused row-sum (accum_out), P stored bf16, normalized by reciprocal
    row-sum on GPSIMD.
  - P@V: P chunks transposed on TensorE (bf16), out^T accumulated in PSUM.
  - output projection: attnT^T @ Wo^T slice (bf16), partial [S, D] fp32.
"""

import math
import sys

import numpy as np

sys.path.insert(0, "/opt/trn_rl_repo")

import ml_dtypes  # noqa: E402

import concourse.bass as bass  # noqa: E402
from concourse import bacc  # noqa: E402
import concourse.mybir as mybir  # noqa: E402
import concourse.tile as tile  # noqa: E402
from concourse import bass_utils  # noqa: E402

F32 = mybir.dt.float32
F32R = mybir.dt.float32r
BF16 = mybir.dt.bfloat16
AF = mybir.ActivationFunctionType
BF16NP = ml_dtypes.bfloat16

B, S, D = 2, 2048, 2048
H_LOCAL = 4          # heads per core
DH = 128
HD_LOCAL = H_LOCAL * DH   # 512
N_CORES = 8
N_SQ = S // 128      # 16 query tiles
N_SB = 4             # s-blocks for projection (512 each)
SBLK = S // N_SB     # 512
KCH = D // 128       # 16 contraction chunks
SCALE = 1.0 / math.sqrt(DH)
NEG = -30000.0

_CACHE = {}
_RUN_KWARGS = {}  # test harness can set e.g. {"trace": True} for profiling


def _build_program():
    if "nc" in _CACHE:
        return _CACHE["nc"]

    nc = bacc.Bacc("TRN2", target_bir_lowering=False, debug=False)

    xT_d = nc.dram_tensor("xT", (D, S), BF16, kind="ExternalInput").ap()
    wqT_d = nc.dram_tensor("wqT", (D, HD_LOCAL), BF16, kind="ExternalInput").ap()
    wkT_d = nc.dram_tensor("wkT", (D, HD_LOCAL), BF16, kind="ExternalInput").ap()
    wvT_d = nc.dram_tensor("wvT", (D, HD_LOCAL), BF16, kind="ExternalInput").ap()
    woT_d = nc.dram_tensor("woT", (HD_LOCAL, D), BF16, kind="ExternalInput").ap()
    cosT_d = nc.dram_tensor("cosT", (DH, S), BF16, kind="ExternalInput").ap()
    sinT_d = nc.dram_tensor("sinT", (DH, S), BF16, kind="ExternalInput").ap()
    jmat_d = nc.dram_tensor("jmatT", (DH, DH), BF16, kind="ExternalInput").ap()
    maskL_d = nc.dram_tensor("maskL", (128, 128), F32, kind="ExternalInput").ap()
    ident_d = nc.dram_tensor("ident", (128, 128), BF16, kind="ExternalInput").ap()
    out_d = nc.dram_tensor("out", (S, D), F32, kind="ExternalOutput").ap()

    with tile.TileContext(nc) as tc:
        with tc.tile_pool(name="resident", bufs=1) as res:
            qt_sb = res.tile([128, H_LOCAL * S], BF16)       # (h, s)
            kt_sb = res.tile([128, H_LOCAL * S], BF16)       # (h, s)
            v_sb = res.tile([128, N_SQ * HD_LOCAL], BF16)    # (s-chunk, hd)
            at_sb = res.tile([128, H_LOCAL * S], BF16)       # attnT (h, s)
            wo_sb = res.tile([128, H_LOCAL * S], BF16)       # (hc, e)
            cos_sb = res.tile([128, S], BF16)
            sin_sb = res.tile([128, S], BF16)
            jmat_sb = res.tile([128, 128], BF16)
            maskL_sb = res.tile([128, 128], F32)
            ident_sb = res.tile([128, 128], BF16)

            nc.sync.dma_start(cos_sb[:], cosT_d[:])
            nc.sync.dma_start(sin_sb[:], sinT_d[:])
            nc.sync.dma_start(jmat_sb[:], jmat_d[:])
            nc.sync.dma_start(maskL_sb[:], maskL_d[:])
            nc.sync.dma_start(ident_sb[:], ident_d[:])
            for hc in range(H_LOCAL):
                nc.sync.dma_start(
                    wo_sb[:, hc * S:(hc + 1) * S], woT_d[hc * 128:(hc + 1) * 128, :]
                )

            # ---------------- projections + RoPE ----------------
            with (
                tc.tile_pool(name="xpool", bufs=2) as xpool,
                tc.tile_pool(name="wpool", bufs=3) as wpool,
                tc.tile_pool(name="pstage", bufs=3) as pstage,
                tc.tile_pool(name="ptmp", bufs=2) as ptmp,
                tc.tile_pool(name="projps", bufs=1, space="PSUM") as projps,
                tc.tile_pool(name="rotps", bufs=2, space="PSUM") as rotps,
                tc.tile_pool(name="vtps", bufs=1, space="PSUM") as vtps,
            ):
                for sb in range(N_SB):
                    xblk = xpool.tile([128, KCH * SBLK], BF16, tag="xblk")
                    for k in range(KCH):
                        nc.sync.dma_start(
                            xblk[:, k * SBLK:(k + 1) * SBLK],
                            xT_d[k * 128:(k + 1) * 128, sb * SBLK:(sb + 1) * SBLK],
                        )
                    for wT_dram, kind in ((wqT_d, "q"), (wkT_d, "k"), (wvT_d, "v")):
                        psums = []
                        for j in range(H_LOCAL):
                            pj = projps.tile([128, SBLK], F32, tag="proj", bufs=5,
                                             name=f"proj_{sb}_{kind}_{j}")
                            psums.append(pj)
                        for k in range(KCH):
                            wt = wpool.tile([128, HD_LOCAL], BF16, tag="w",
                                            name=f"w_{sb}_{kind}_{k}")
                            nc.sync.dma_start(wt[:], wT_dram[k * 128:(k + 1) * 128, :])
                            for j in range(H_LOCAL):
                                nc.tensor.matmul(
                                    psums[j][:],
                                    wt[:, j * 128:(j + 1) * 128],
                                    xblk[:, k * SBLK:(k + 1) * SBLK],
                                    start=(k == 0),
                                    stop=(k == KCH - 1),
                                )
                        for j in range(H_LOCAL):
                            if kind in ("q", "k"):
                                # RoPE: q' = q*cos + (J q)*sin, J applied on PE
                                qst = pstage.tile([128, SBLK], BF16, tag="qst",
                                                  name=f"qst_{sb}_{kind}_{j}")
                                nc.scalar.copy(qst[:], psums[j][:])
                                rps = rotps.tile([128, SBLK], F32, tag="rot",
                                                 name=f"rot_{sb}_{kind}_{j}")
                                nc.tensor.matmul(
                                    rps[:], jmat_sb[:], qst[:], start=True, stop=True,
                                )
                                m1 = ptmp.tile([128, SBLK], F32, tag="m1",
                                               name=f"m1_{sb}_{kind}_{j}")
                                nc.vector.tensor_mul(
                                    m1[:], qst[:], cos_sb[:, sb * SBLK:(sb + 1) * SBLK]
                                )
                                m2 = ptmp.tile([128, SBLK], F32, tag="m2",
                                               name=f"m2_{sb}_{kind}_{j}")
                                nc.vector.tensor_mul(
                                    m2[:], rps[:], sin_sb[:, sb * SBLK:(sb + 1) * SBLK]
                                )
                                dest = qt_sb if kind == "q" else kt_sb
                                nc.vector.tensor_add(
                                    dest[:, j * S + sb * SBLK: j * S + (sb + 1) * SBLK],
                                    m1[:], m2[:],
                                )
                            else:
                                # V^T -> V via TensorE transpose (bf16)
                                vst = pstage.tile([128, SBLK], BF16, tag="vst",
                                                  name=f"vst_{sb}_{j}")
                                nc.scalar.copy(vst[:], psums[j][:])
                                for ss in range(SBLK // 128):
                                    vt = vtps.tile([128, 128], BF16, tag="vt",
                                                   bufs=1, name=f"vt_{sb}_{j}_{ss}")
                                    nc.tensor.transpose(
                                        vt[:], vst[:, ss * 128:(ss + 1) * 128],
                                        ident_sb[:],
                                    )
                                    c = sb * (SBLK // 128) + ss
                                    nc.scalar.copy(
                                        v_sb[:, c * HD_LOCAL + j * 128:
                                             c * HD_LOCAL + (j + 1) * 128],
                                        vt[:],
                                    )

            # ------------- attention + fused output projection -------------
            with (
                tc.tile_pool(name="ppool", bufs=3) as ppool,
                tc.tile_pool(name="accpool", bufs=8) as accpool,
                tc.tile_pool(name="ptsb", bufs=10) as ptsbp,
                tc.tile_pool(name="osb", bufs=3) as osbp,
                tc.tile_pool(name="scps", bufs=2, space="PSUM") as scps,
                tc.tile_pool(name="ptps", bufs=3, space="PSUM") as ptps,
                tc.tile_pool(name="ops", bufs=1, space="PSUM") as ops,
                tc.tile_pool(name="fps", bufs=2, space="PSUM") as fps,
            ):
                for t in range(N_SQ):
                    for h in range(H_LOCAL):
                        n = t + 1  # causal: chunks 0..t
                        nblk = (n + 3) // 4
                        p_t = ppool.tile([128, N_SQ * 128], BF16, tag="p",
                                         name=f"p_{h}_{t}")
                        accs = []
                        for bi in range(nblk):
                            w = min(4, n - 4 * bi) * 128
                            sps = scps.tile([128, 512], F32, tag="sc",
                                            name=f"sc_{h}_{t}_{bi}")
                            nc.tensor.matmul(
                                sps[:, :w],
                                qt_sb[:, h * S + t * 128: h * S + (t + 1) * 128],
                                kt_sb[:, h * S + bi * 512: h * S + bi * 512 + w],
                                start=True, stop=True,
                            )
                            # additive causal mask on the diagonal chunk (c == t)
                            if t // 4 == bi:
                                off = (t % 4) * 128
                                nc.vector.tensor_add(
                                    sps[:, off:off + 128], sps[:, off:off + 128],
                                    maskL_sb[:],
                                )
                            acc = accpool.tile([128, 1], F32, tag="acc",
                                               name=f"acc_{h}_{t}_{bi}")
                            nc.scalar.activation(
                                p_t[:, bi * 512: bi * 512 + w], sps[:, :w],
                                AF.Exp, scale=SCALE, accum_out=acc[:],
                            )
                            accs.append(acc)
                        rs = accs[0]
                        for acc in accs[1:]:
                            rs2 = accpool.tile([128, 1], F32, tag="rss",
                                               name=f"rs_{h}_{t}")
                            nc.vector.tensor_add(rs2[:], rs[:], acc[:])
                            rs = rs2
                        rcp = accpool.tile([128, 1], F32, tag="rcp",
                                           name=f"rcp_{h}_{t}")
                        nc.vector.reciprocal(rcp[:], rs[:])
                        nc.scalar.mul(p_t[:, :n * 128], p_t[:, :n * 128], rcp[:])
                        opsum = ops.tile([128, 128], F32, tag="o",
                                         name=f"o_{h}_{t}")
                        for c in range(n):
                            ptp = ptps.tile([128, 128], BF16, tag="pt",
                                            name=f"pt_{h}_{t}_{c}")
                            nc.tensor.transpose(
                                ptp[:], p_t[:, c * 128:(c + 1) * 128], ident_sb[:]
                            )
                            pts = ptsbp.tile([128, 128], BF16, tag="pts",
                                             name=f"pts_{h}_{t}_{c}")
                            nc.vector.tensor_copy(pts[:], ptp[:])
                            nc.tensor.matmul(
                                opsum[:],
                                v_sb[:, c * HD_LOCAL + h * 128:
                                     c * HD_LOCAL + (h + 1) * 128],
                                pts[:],
                                start=(c == 0), stop=(c == n - 1),
                            )
                        nc.scalar.copy(
                            at_sb[:, h * S + t * 128: h * S + (t + 1) * 128],
                            opsum[:],
                        )
                    # output projection for s-tile t (all 4 heads now ready)
                    for e in range(D // 512):
                        fp = fps.tile([128, 512], F32, tag="f",
                                      name=f"f_{t}_{e}")
                        for hc in range(H_LOCAL):
                            nc.tensor.matmul(
                                fp[:],
                                at_sb[:, hc * S + t * 128: hc * S + (t + 1) * 128],
                                wo_sb[:, hc * S + e * 512: hc * S + (e + 1) * 512],
                                start=(hc == 0), stop=(hc == H_LOCAL - 1),
                            )
                        ob = osbp.tile([128, 512], F32, tag="ob",
                                       name=f"ob_{t}_{e}")
                        nc.vector.tensor_copy(ob[:], fp[:])
                        nc.sync.dma_start(
                            out_d[t * 128:(t + 1) * 128, e * 512:(e + 1) * 512],
                            ob[:],
                        )

    nc.compile()
    _CACHE["nc"] = nc
    return nc


def _host_constants():
    if "consts" in _CACHE:
        return _CACHE["consts"]
    inv = 1.0 / (10000.0 ** (np.arange(0, DH, 2, dtype=np.float32) / DH))
    t = np.arange(S, dtype=np.float32)
    freqs = np.outer(t, inv)                       # [S, 64]
    emb = np.concatenate([freqs, freqs], -1)       # [S, 128]
    cosT = np.cos(emb).T.astype(BF16NP).copy()     # [128, S]
    sinT = np.sin(emb).T.astype(BF16NP).copy()
    j = np.zeros((DH, DH), dtype=np.float32)
    half = DH // 2
    for p in range(half):
        j[p, p + half] = -1.0
        j[p + half, p] = 1.0
    jmatT = j.T.astype(BF16NP).copy()                             # lhsT so that lhsT.T = J
    r = np.arange(128)
    maskL = np.where(r[None, :] <= r[:, None], 0.0, NEG).astype(np.float32)
    ident = np.eye(128, dtype=BF16NP)
    _CACHE["consts"] = (cosT, sinT, jmatT, maskL, ident)
    return _CACHE["consts"]


def kernel(hidden_states, Wq, Wk, Wv, Wo):
    hidden_states = np.asarray(hidden_states, dtype=np.float32)
    Wq = np.asarray(Wq, dtype=np.float32)
    Wk = np.asarray(Wk, dtype=np.float32)
    Wv = np.asarray(Wv, dtype=np.float32)
    Wo = np.asarray(Wo, dtype=np.float32)

    nc = _build_program()
    cosT, sinT, jmatT, maskL, ident = _host_constants()

    in_maps = []
    for core in range(N_CORES):
        b, g = divmod(core, N_CORES // B)
        hd0 = g * HD_LOCAL
        in_maps.append({
            "xT": np.ascontiguousarray(hidden_states[b].T).astype(BF16NP),
            "wqT": np.ascontiguousarray(Wq[hd0:hd0 + HD_LOCAL, :].T).astype(BF16NP),
            "wkT": np.ascontiguousarray(Wk[hd0:hd0 + HD_LOCAL, :].T).astype(BF16NP),
            "wvT": np.ascontiguousarray(Wv[hd0:hd0 + HD_LOCAL, :].T).astype(BF16NP),
            "woT": np.ascontiguousarray(Wo[:, hd0:hd0 + HD_LOCAL].T).astype(BF16NP),
            "cosT": cosT,
            "sinT": sinT,
            "jmatT": jmatT,
            "maskL": maskL,
            "ident": ident,
        })

    res = bass_utils.run_bass_kernel_spmd(
        nc, in_maps, core_ids=list(range(N_CORES)), **_RUN_KWARGS
    )
    _CACHE["last_results"] = res

    out = np.zeros((B, S, D), dtype=np.float32)
    for core in range(N_CORES):
        b = core // (N_CORES // B)
        out[b] += res.results[core]["out"]
    return out


# revision 11
# speedup vs baseline: 3.1557x; 1.0405x over previous
"""Trainium2 Bass kernel for nn_MultiHeadAttention_82523501625370.

Multi-head attention (B=2, S=2048, D=2048, H=16, Dh=128) with RoPE and a
causal mask (the reference's sliding-window clause `(j-i) >= 1024` is a
subset of the causal clause `j > i`, so the effective mask is plain causal).

Sharding: 8 cores = 2 batches x 4 head-groups (4 heads each).
Per core: column-parallel Wq/Wk/Wv (512 output dims), local attention for
its 4 heads, row-parallel Wo slice -> partial [S, D] output, summed on host.

On-chip dataflow (per core):
  - projections: Q^T/K^T/V^T tiles via fp32r matmuls (W^T chunks streamed,
    x^T s-block resident), RoPE applied via a constant J-permutation matmul
    plus DVE multiplies, results stored bf16.
  - V^T transposed to V [s, dh] layout via TensorE (bf16).
  - scores: S[sq,sk] = Q^T.T @ K^T per 128-row query tile, additive causal
    mask on the diagonal chunk, exp on ScalarE with fused 1/sqrt(dh) scale
    and fused row-sum (accum_out), P stored bf16, normalized by reciprocal
    row-sum on GPSIMD.
  - P@V: P chunks transposed on TensorE (bf16), out^T accumulated in PSUM.
  - output projection: attnT^T @ Wo^T slice (bf16), partial [S, D] fp32.
"""

import math
import sys

import numpy as np

sys.path.insert(0, "/opt/trn_rl_repo")

import ml_dtypes  # noqa: E402

import concourse.bass as bass  # noqa: E402
from concourse import bacc  # noqa: E402
import concourse.mybir as mybir  # noqa: E402
import concourse.tile as tile  # noqa: E402
from concourse import bass_utils  # noqa: E402

F32 = mybir.dt.float32
F32R = mybir.dt.float32r
BF16 = mybir.dt.bfloat16
AF = mybir.ActivationFunctionType
BF16NP = ml_dtypes.bfloat16

B, S, D = 2, 2048, 2048
H_LOCAL = 4          # heads per core
DH = 128
HD_LOCAL = H_LOCAL * DH   # 512
N_CORES = 8
N_SQ = S // 128      # 16 query tiles
N_SB = 4             # s-blocks for projection (512 each)
SBLK = S // N_SB     # 512
KCH = D // 128       # 16 contraction chunks
SCALE = 1.0 / math.sqrt(DH)
NEG = -30000.0

_CACHE = {}
_RUN_KWARGS = {}  # test harness can set e.g. {"trace": True} for profiling


def _build_program():
    if "nc" in _CACHE:
        return _CACHE["nc"]

    nc = bacc.Bacc("TRN2", target_bir_lowering=False, debug=False)

    xT_d = nc.dram_tensor("xT", (D, S), BF16, kind="ExternalInput").ap()
    wqT_d = nc.dram_tensor("wqT", (D, HD_LOCAL), BF16, kind="ExternalInput").ap()
    wkT_d = nc.dram_tensor("wkT", (D, HD_LOCAL), BF16, kind="ExternalInput").ap()
    wvT_d = nc.dram_tensor("wvT", (D, HD_LOCAL), BF16, kind="ExternalInput").ap()
    woT_d = nc.dram_tensor("woT", (HD_LOCAL, D), BF16, kind="ExternalInput").ap()
    cosT_d = nc.dram_tensor("cosT", (DH, S), BF16, kind="ExternalInput").ap()
    sinT_d = nc.dram_tensor("sinT", (DH, S), BF16, kind="ExternalInput").ap()
    jmat_d = nc.dram_tensor("jmatT", (DH, DH), BF16, kind="ExternalInput").ap()
    maskL_d = nc.dram_tensor("maskL", (128, 128), F32, kind="ExternalInput").ap()
    ident_d = nc.dram_tensor("ident", (128, 128), BF16, kind="ExternalInput").ap()
    out_d = nc.dram_tensor("out", (S, D), F32, kind="ExternalOutput").ap()

    with tile.TileContext(nc) as tc:
        with tc.tile_pool(name="resident", bufs=1) as res:
            qt_sb = res.tile([128, H_LOCAL * S], BF16)       # (h, s)
            kt_sb = res.tile([128, H_LOCAL * S], BF16)       # (h, s)
            v_sb = res.tile([128, N_SQ * HD_LOCAL], BF16)    # (s-chunk, hd)
            at_sb = res.tile([128, H_LOCAL * S], BF16)       # attnT (h, s)
            wo_sb = res.tile([128, H_LOCAL * S], BF16)       # (hc, e)
            cos_sb = res.tile([128, S], BF16)
            sin_sb = res.tile([128, S], BF16)
            jmat_sb = res.tile([128, 128], BF16)
            maskL_sb = res.tile([128, 128], F32)
            ident_sb = res.tile([128, 128], BF16)

            nc.sync.dma_start(cos_sb[:], cosT_d[:])
            nc.sync.dma_start(sin_sb[:], sinT_d[:])
            nc.sync.dma_start(jmat_sb[:], jmat_d[:])
            nc.sync.dma_start(maskL_sb[:], maskL_d[:])
            nc.sync.dma_start(ident_sb[:], ident_d[:])
            for hc in range(H_LOCAL):
                nc.sync.dma_start(
                    wo_sb[:, hc * S:(hc + 1) * S], woT_d[hc * 128:(hc + 1) * 128, :]
                )

            # ---------------- projections + RoPE ----------------
            with (
                tc.tile_pool(name="xpool", bufs=2) as xpool,
                tc.tile_pool(name="wpool", bufs=3) as wpool,
                tc.tile_pool(name="pstage", bufs=3) as pstage,
                tc.tile_pool(name="ptmp", bufs=2) as ptmp,
                tc.tile_pool(name="projps", bufs=1, space="PSUM") as projps,
                tc.tile_pool(name="rotps", bufs=2, space="PSUM") as rotps,
            ):
                for sb in range(N_SB):
                    xblk = xpool.tile([128, KCH * SBLK], BF16, tag="xblk")
                    for k in range(KCH):
                        nc.sync.dma_start(
                            xblk[:, k * SBLK:(k + 1) * SBLK],
                            xT_d[k * 128:(k + 1) * 128, sb * SBLK:(sb + 1) * SBLK],
                        )
                    for wT_dram, kind in ((wqT_d, "q"), (wkT_d, "k"), (wvT_d, "v")):
                        psums = []
                        for j in range(H_LOCAL):
                            pj = projps.tile([128, SBLK], F32, tag="proj", bufs=5,
                                             name=f"proj_{sb}_{kind}_{j}")
                            psums.append(pj)
                        for k in range(KCH):
                            wt = wpool.tile([128, HD_LOCAL], BF16, tag="w",
                                            name=f"w_{sb}_{kind}_{k}")
                            nc.sync.dma_start(wt[:], wT_dram[k * 128:(k + 1) * 128, :])
                            for j in range(H_LOCAL):
                                if kind == "v":
                                    # V in natural [s, hd] layout: x subtile stationary
                                    nc.tensor.matmul(
                                        psums[j][:],
                                        xblk[:, k * SBLK + j * 128:
                                             k * SBLK + (j + 1) * 128],
                                        wt[:],
                                        start=(k == 0),
                                        stop=(k == KCH - 1),
                                    )
                                else:
                                    nc.tensor.matmul(
                                        psums[j][:],
                                        wt[:, j * 128:(j + 1) * 128],
                                        xblk[:, k * SBLK:(k + 1) * SBLK],
                                        start=(k == 0),
                                        stop=(k == KCH - 1),
                                    )
                        for j in range(H_LOCAL):
                            if kind in ("q", "k"):
                                # RoPE: q' = q*cos + (J q)*sin, J applied on PE
                                qst = pstage.tile([128, SBLK], BF16, tag="qst",
                                                  name=f"qst_{sb}_{kind}_{j}")
                                nc.scalar.copy(qst[:], psums[j][:])
                                rps = rotps.tile([128, SBLK], F32, tag="rot",
                                                 name=f"rot_{sb}_{kind}_{j}")
                                nc.tensor.matmul(
                                    rps[:], jmat_sb[:], qst[:], start=True, stop=True,
                                )
                                m1 = ptmp.tile([128, SBLK], F32, tag="m1",
                                               name=f"m1_{sb}_{kind}_{j}")
                                nc.vector.tensor_mul(
                                    m1[:], qst[:], cos_sb[:, sb * SBLK:(sb + 1) * SBLK]
                                )
                                m2 = ptmp.tile([128, SBLK], F32, tag="m2",
                                               name=f"m2_{sb}_{kind}_{j}")
                                nc.vector.tensor_mul(
                                    m2[:], rps[:], sin_sb[:, sb * SBLK:(sb + 1) * SBLK]
                                )
                                dest = qt_sb if kind == "q" else kt_sb
                                nc.vector.tensor_add(
                                    dest[:, j * S + sb * SBLK: j * S + (sb + 1) * SBLK],
                                    m1[:], m2[:],
                                )
                            else:
                                c = sb * (SBLK // 128) + j
                                nc.scalar.copy(
                                    v_sb[:, c * HD_LOCAL:(c + 1) * HD_LOCAL],
                                    psums[j][:],
                                )

            # ------------- attention + fused output projection -------------
            with (
                tc.tile_pool(name="ppool", bufs=3) as ppool,
                tc.tile_pool(name="accpool", bufs=8) as accpool,
                tc.tile_pool(name="ptsb", bufs=4) as ptsbp,
                tc.tile_pool(name="osb", bufs=3) as osbp,
                tc.tile_pool(name="scps", bufs=2, space="PSUM") as scps,
                tc.tile_pool(name="ptps", bufs=3, space="PSUM") as ptps,
                tc.tile_pool(name="ops", bufs=1, space="PSUM") as ops,
                tc.tile_pool(name="fps", bufs=2, space="PSUM") as fps,
            ):
                for t in range(N_SQ):
                    for h in range(H_LOCAL):
                        n = t + 1  # causal: chunks 0..t
                        nblk = (n + 3) // 4
                        p_t = ppool.tile([128, N_SQ * 128], BF16, tag="p",
                                         name=f"p_{h}_{t}")
                        accs = []
                        for bi in range(nblk):
                            w = min(4, n - 4 * bi) * 128
                            sps = scps.tile([128, 512], F32, tag="sc",
                                            name=f"sc_{h}_{t}_{bi}")
                            nc.tensor.matmul(
                                sps[:, :w],
                                qt_sb[:, h * S + t * 128: h * S + (t + 1) * 128],
                                kt_sb[:, h * S + bi * 512: h * S + bi * 512 + w],
                                start=True, stop=True,
                            )
                            # additive causal mask on the diagonal chunk (c == t)
                            if t // 4 == bi:
                                off = (t % 4) * 128
                                nc.vector.tensor_add(
                                    sps[:, off:off + 128], sps[:, off:off + 128],
                                    maskL_sb[:],
                                )
                            acc = accpool.tile([128, 1], F32, tag="acc",
                                               name=f"acc_{h}_{t}_{bi}")
                            nc.scalar.activation(
                                p_t[:, bi * 512: bi * 512 + w], sps[:, :w],
                                AF.Exp, scale=SCALE, accum_out=acc[:],
                            )
                            accs.append(acc)
                        rs = accs[0]
                        for acc in accs[1:]:
                            rs2 = accpool.tile([128, 1], F32, tag="rss",
                                               name=f"rs_{h}_{t}")
                            nc.vector.tensor_add(rs2[:], rs[:], acc[:])
                            rs = rs2
                        rcp = accpool.tile([128, 1], F32, tag="rcp",
                                           name=f"rcp_{h}_{t}")
                        nc.vector.reciprocal(rcp[:], rs[:])
                        nc.scalar.mul(p_t[:, :n * 128], p_t[:, :n * 128], rcp[:])
                        opsum = ops.tile([128, 128], F32, tag="o",
                                         name=f"o_{h}_{t}")
                        for g in range(nblk):
                            gn = min(4, n - 4 * g)
                            ptp = ptps.tile([128, 512], BF16, tag="pt",
                                            name=f"pt_{h}_{t}_{g}")
                            for ci in range(gn):
                                c = 4 * g + ci
                                nc.tensor.transpose(
                                    ptp[:, ci * 128:(ci + 1) * 128],
                                    p_t[:, c * 128:(c + 1) * 128], ident_sb[:]
                                )
                            pts = ptsbp.tile([128, 512], BF16, tag="pts",
                                             name=f"pts_{h}_{t}_{g}")
                            nc.vector.tensor_copy(pts[:, :gn * 128],
                                                  ptp[:, :gn * 128])
                            for ci in range(gn):
                                c = 4 * g + ci
                                nc.tensor.matmul(
                                    opsum[:],
                                    v_sb[:, c * HD_LOCAL + h * 128:
                                         c * HD_LOCAL + (h + 1) * 128],
                                    pts[:, ci * 128:(ci + 1) * 128],
                                    start=(c == 0), stop=(c == n - 1),
                                )
                        nc.scalar.copy(
                            at_sb[:, h * S + t * 128: h * S + (t + 1) * 128],
                            opsum[:],
                        )
                    # output projection for s-tile t (all 4 heads now ready)
                    for e in range(D // 512):
                        fp = fps.tile([128, 512], F32, tag="f",
                                      name=f"f_{t}_{e}")
                        for hc in range(H_LOCAL):
                            nc.tensor.matmul(
                                fp[:],
                                at_sb[:, hc * S + t * 128: hc * S + (t + 1) * 128],
                                wo_sb[:, hc * S + e * 512: hc * S + (e + 1) * 512],
                                start=(hc == 0), stop=(hc == H_LOCAL - 1),
                            )
                        ob = osbp.tile([128, 512], F32, tag="ob",
                                       name=f"ob_{t}_{e}")
                        nc.vector.tensor_copy(ob[:], fp[:])
                        nc.sync.dma_start(
                            out_d[t * 128:(t + 1) * 128, e * 512:(e + 1) * 512],
                            ob[:],
                        )

    nc.compile()
    _CACHE["nc"] = nc
    return nc


def _host_constants():
    if "consts" in _CACHE:
        return _CACHE["consts"]
    inv = 1.0 / (10000.0 ** (np.arange(0, DH, 2, dtype=np.float32) / DH))
    t = np.arange(S, dtype=np.float32)
    freqs = np.outer(t, inv)                       # [S, 64]
    emb = np.concatenate([freqs, freqs], -1)       # [S, 128]
    cosT = np.cos(emb).T.astype(BF16NP).copy()     # [128, S]
    sinT = np.sin(emb).T.astype(BF16NP).copy()
    j = np.zeros((DH, DH), dtype=np.float32)
    half = DH // 2
    for p in range(half):
        j[p, p + half] = -1.0
        j[p + half, p] = 1.0
    jmatT = j.T.astype(BF16NP).copy()                             # lhsT so that lhsT.T = J
    r = np.arange(128)
    maskL = np.where(r[None, :] <= r[:, None], 0.0, NEG).astype(np.float32)
    ident = np.eye(128, dtype=BF16NP)
    _CACHE["consts"] = (cosT, sinT, jmatT, maskL, ident)
    return _CACHE["consts"]


def kernel(hidden_states, Wq, Wk, Wv, Wo):
    hidden_states = np.asarray(hidden_states, dtype=np.float32)
    Wq = np.asarray(Wq, dtype=np.float32)
    Wk = np.asarray(Wk, dtype=np.float32)
    Wv = np.asarray(Wv, dtype=np.float32)
    Wo = np.asarray(Wo, dtype=np.float32)

    nc = _build_program()
    cosT, sinT, jmatT, maskL, ident = _host_constants()

    in_maps = []
    for core in range(N_CORES):
        b, g = divmod(core, N_CORES // B)
        hd0 = g * HD_LOCAL
        in_maps.append({
            "xT": np.ascontiguousarray(hidden_states[b].T).astype(BF16NP),
            "wqT": np.ascontiguousarray(Wq[hd0:hd0 + HD_LOCAL, :].T).astype(BF16NP),
            "wkT": np.ascontiguousarray(Wk[hd0:hd0 + HD_LOCAL, :].T).astype(BF16NP),
            "wvT": np.ascontiguousarray(Wv[hd0:hd0 + HD_LOCAL, :].T).astype(BF16NP),
            "woT": np.ascontiguousarray(Wo[:, hd0:hd0 + HD_LOCAL].T).astype(BF16NP),
            "cosT": cosT,
            "sinT": sinT,
            "jmatT": jmatT,
            "maskL": maskL,
            "ident": ident,
        })

    res = bass_utils.run_bass_kernel_spmd(
        nc, in_maps, core_ids=list(range(N_CORES)), **_RUN_KWARGS
    )
    _CACHE["last_results"] = res

    out = np.zeros((B, S, D), dtype=np.float32)
    for core in range(N_CORES):
        b = core // (N_CORES // B)
        out[b] += res.results[core]["out"]
    return out
